# revision 21
# baseline (speedup 1.0000x reference)
"""Trainium2 Bass kernel for the DCM sparse-attention problem (v4, bf16).

Same math restructure as the baseline (S-matrix collapse: every softmax
aggregation is a weighted reduction of S[(a,t),(b,v)] = <t,v>/|t||v|),
plus:

- All matmuls in bf16 (1 cyc/row; DMA bytes halved vs f32).
- Video-norm fold AFTER the S matmul (S matmuls start as soon as tiles
  land); tau and r_t fold into that same scalar_tensor_tensor pass, the
  text mask becomes the E-exponential's per-partition scale, and the
  final division absorbs the leftover tau.
- rsqrt = exp(-0.5*ln(x)); Ln and Exp are steered into the one PWP
  table set that holds both, so there is a single hoisted table load
  and no mid-pipeline table switch.
- Warm-up/keepalive junk matmuls hold the PE HAM clock gate open across
  the DMA window and the elementwise mid-section.
- The mid-section is issued in (M-tile, column-half) streams so DVE and
  ACT pipeline instead of serializing on full-width tensors.
- GpSimd does only the mask/const DMAs (SWDGE) and the one rv
  partition-broadcast; its tensor ops are too slow (drains).

Each of the 8 cores handles 8 of the 64 text rows (A-sharded, video
replicated).
"""

import sys

sys.path.insert(0, "/opt/trn_rl_repo")

import ml_dtypes
import numpy as np

import concourse.bass as bass
import concourse.bacc as bacc
import concourse.hw_specs as hw_specs
import concourse.tile as tile
from concourse import mybir
from concourse.bass_utils import run_bass_kernel_spmd

TAU = 100.0
A, T, B, V, D = 64, 32, 64, 12, 512
NCORES = 8
AL = A // NCORES          # a's per core = 8
AT = AL * T               # (a,t) rows per core = 256
BV = B * V                # (b,v) cols = 768
NMT = AT // 128           # M-tiles over (a,t) = 2
NKT = D // 128            # K-tiles over d = 4
APB = 128 // T            # a's per M-tile = 4
F32 = mybir.dt.float32
BF16 = mybir.dt.bfloat16
EXP = mybir.ActivationFunctionType.Exp
LN = mybir.ActivationFunctionType.Ln
MUL = mybir.AluOpType.mult
X = mybir.AxisListType.X
NSL = [(0, 512), (512, 768)]                   # bank-aligned slices of 768
NSL3 = [(0, 512), (512, 1024), (1024, 1536)]   # ... of 1536
HALF = [(0, 384), (384, 768)]                  # group-aligned halves
WSL = [(0, 384), (384, 512), (512, 768)]       # bank-safe W4 chunks
NWARM = 4

_JOINT = "natural_log_exp_and_others"
_orig_gat = hw_specs.get_activation_tables


def _gat(arch):
    """Steer Ln and Exp to the one table set containing both, so the
    activation-table pass emits a single load instead of three.  Set ids
    are positional, so entries are filtered in place, never reordered."""
    tables = _orig_gat(arch)
    if _JOINT in tables:
        for name, funcs in tables.items():
            if name != _JOINT:
                funcs.discard(LN)
                funcs.discard(EXP)
    return tables


bacc.get_activation_tables = _gat


def _build_program():
    nc = bacc.Bacc("TRN2", target_bir_lowering=False)

    tT_d = nc.declare_dram_parameter("tT", [128, NKT * AT], BF16, isOutput=False)
    vT_d = nc.declare_dram_parameter("vT", [128, NKT * BV], BF16, isOutput=False)
    mask_d = nc.declare_dram_parameter("mask", [128, NMT], F32, isOutput=False)
    cpack_d = nc.declare_dram_parameter("cpack", [128, NMT * 8 + 1], BF16,
                                        isOutput=False)
    indW_d = nc.declare_dram_parameter("indW", [8, NMT * 128], BF16,
                                       isOutput=False)
    out_d = nc.declare_dram_parameter("out", [AL, B], F32, isOutput=True)

    with tile.TileContext(nc) as tc:
        with (
            tc.tile_pool(name="consts", bufs=1) as consts,
            tc.tile_pool(name="inputs", bufs=1) as inputs,
            tc.tile_pool(name="sq", bufs=1) as sqp,
            tc.tile_pool(name="big", bufs=1) as bigp,
            tc.tile_pool(name="smalls", bufs=1) as smalls,
            tc.tile_pool(name="psA", bufs=2, space="PSUM") as psA,
            tc.tile_pool(name="psB", bufs=1, space="PSUM") as psB,
        ):
            # ---- PE warm-up: junk matmuls feed the HAM activity monitor
            # during the DMA window so real matmuls run at released clock ----
            junk = consts.tile([128, 512], BF16)
            nc.vector.memset(junk, 1.0)
            ps_warm = psA.tile([128, 512], F32, tag="s")
            for w in range(NWARM):
                nc.tensor.matmul(ps_warm, junk[:, 0:128], junk,
                                 start=True, stop=True)

            # ---- input DMAs: vT per-k on sync, tT halves on scalar,
            # consts/mask on the idle gpsimd SWDGE queue ----
            vT = inputs.tile([128, NKT * BV], BF16)
            tT = inputs.tile([128, NKT * AT], BF16)
            for k in range(2):
                nc.sync.dma_start(out=vT[:, k * BV:(k + 1) * BV],
                                  in_=vT_d[:, k * BV:(k + 1) * BV])
            nc.scalar.dma_start(out=tT[:, :2 * AT], in_=tT_d[:, :2 * AT])
            nc.scalar.dma_start(out=tT[:, 2 * AT:], in_=tT_d[:, 2 * AT:])
            nc.scalar.dma_start(out=vT[:, 3 * BV:], in_=vT_d[:, 3 * BV:])
            maskt = consts.tile([128, NMT], F32)
            nc.gpsimd.dma_start(out=maskt, in_=mask_d[:, :])
            nc.gpsimd.dma_start(out=vT[:, 2 * BV:3 * BV],
                                in_=vT_d[:, 2 * BV:3 * BV])
            cpack = consts.tile([128, NMT * 8 + 1], BF16)
            nc.gpsimd.dma_start(out=cpack, in_=cpack_d[:, :])
            indW = consts.tile([8, NMT * 128], BF16)
            nc.gpsimd.dma_start(out=indW, in_=indW_d[:, :])
            ind36 = cpack[:, :NMT * 8]
            onesc = cpack[:, NMT * 8:]

            # ---- per-k: square (DVE), norm ones-matmuls, then S matmuls;
            # each k-tile is consumed as its DMA lands ----
            # combined [sqv | sqt] layout: the norm needs only two
            # bank-aligned ones-matmuls per k instead of three
            sq = sqp.tile([128, NKT * (BV + AT)], BF16)
            KW = BV + AT
            ps_n = psB.tile([1, KW], F32, tag="v")
            ps_s = [psA.tile([128, BV], F32, tag="s", name=f"ps_s{i}")
                    for i in range(NMT)]
            for k in range(NKT):
                nc.vector.tensor_tensor(sq[:, k * KW:k * KW + BV],
                                        vT[:, k * BV:(k + 1) * BV],
                                        vT[:, k * BV:(k + 1) * BV], op=MUL)
                nc.vector.tensor_tensor(sq[:, k * KW + BV:(k + 1) * KW],
                                        tT[:, k * AT:(k + 1) * AT],
                                        tT[:, k * AT:(k + 1) * AT], op=MUL)
                for lo, hi in ((0, 512), (512, KW)):
                    nc.tensor.matmul(ps_n[:, lo:hi], onesc,
                                     sq[:, k * KW + lo:k * KW + hi],
                                     start=(k == 0), stop=(k == NKT - 1))
                for i in range(NMT):
                    for lo, hi in NSL:
                        nc.tensor.matmul(
                            ps_s[i][:, lo:hi],
                            tT[:, k * AT + 128 * i:k * AT + 128 * (i + 1)],
                            vT[:, k * BV + lo:k * BV + hi],
                            start=(k == 0), stop=(k == NKT - 1))

            # ---- norms: rsqrt = exp(-0.5*ln(ss)); v-row first so the
            # broadcast starts as early as possible ----
            lss = smalls.tile([1, BV + AT], F32)
            rr = smalls.tile([1, BV + AT], F32)
            rv_bc = bigp.tile([128, BV], F32)
            for lo, hi in HALF:
                nc.scalar.activation(lss[:, lo:hi], ps_n[:, lo:hi], LN)
                nc.scalar.activation(rr[:, lo:hi], lss[:, lo:hi], EXP,
                                     scale=-0.5)
                nc.gpsimd.partition_broadcast(rv_bc[:, lo:hi], rr[:, lo:hi],
                                              channels=128)
            nc.scalar.activation(lss[:, BV:], ps_n[:, BV:], LN)
            nc.scalar.activation(rr[:, BV:], lss[:, BV:], EXP, scale=-0.5)
            ident = consts.tile([1, 1], F32)
            nc.vector.memset(ident, 1.0)
            tau_rt = [smalls.tile([128, 1], F32, name=f"tau_rt{i}")
                      for i in range(NMT)]
            ind36m = [smalls.tile([128, 8], BF16, name=f"ind36m{i}")
                      for i in range(NMT)]
            for i in range(NMT):
                ps_tr = psB.tile([128, 1], F32, tag="j", name=f"ps_tr{i}")
                nc.tensor.transpose(ps_tr,
                                    rr[:, BV + 128 * i:BV + 128 * (i + 1)],
                                    ident)
                nc.vector.tensor_scalar_mul(tau_rt[i], ps_tr, TAU)
                nc.vector.tensor_scalar_mul(ind36m[i],
                                            ind36[:, 8 * i:8 * (i + 1)],
                                            maskt[:, i:i + 1])

            # ---- mid section in (i, half) streams: sp = tau*r_t*rv*S from
            # PSUM, E = exp(mask*sp), ES = sp*E, then grouped reduces ----
            sp = [bigp.tile([128, BV], BF16, name=f"sp{i}") for i in range(NMT)]
            big = [bigp.tile([128, 2 * BV], BF16, name=f"big{i}")
                   for i in range(NMT)]
            rhs_f = [smalls.tile([128, 128], BF16, name=f"rhs_f{i}")
                     for i in range(NMT)]
            red = [smalls.tile([128, 128], F32, name=f"red{i}")
                   for i in range(NMT)]
            for i in range(NMT):
                for lo, hi in HALF:
                    nc.vector.scalar_tensor_tensor(
                        sp[i][:, lo:hi], ps_s[i][:, lo:hi], tau_rt[i],
                        rv_bc[:, lo:hi], op0=MUL, op1=MUL)
                    nc.scalar.activation(big[i][:, BV + lo:BV + hi],
                                         sp[i][:, lo:hi], EXP,
                                         scale=maskt[:, i:i + 1])
            for i in range(NMT):
                nc.vector.reduce_sum(red[i][:, B:],
                                     big[i][:, BV:].rearrange(
                                         "p (g v) -> p g v", v=V), axis=X)
                for lo, hi in HALF:
                    nc.vector.tensor_tensor(big[i][:, lo:hi], sp[i][:, lo:hi],
                                            big[i][:, BV + lo:BV + hi], op=MUL)
                nc.vector.reduce_sum(red[i][:, :B],
                                     big[i][:, :BV].rearrange(
                                         "p (g v) -> p g v", v=V), axis=X)
            for i in range(NMT):
                rdn = smalls.tile([128, B], F32, name=f"rdn{i}")
                nc.vector.reciprocal_approx_fast(rdn, red[i][:, B:])
                t2v = smalls.tile([128, B], F32, name=f"t2v{i}")
                nc.vector.tensor_tensor(t2v, red[i][:, :B], rdn, op=MUL)
                nc.scalar.activation(rhs_f[i][:, B:], t2v, EXP)

            # ---- PE keepalive while DVE/ACT chew the mid-section ----
            for w in range(3):
                nc.tensor.matmul(ps_n[:, 0:512], onesc, junk,
                                 start=True, stop=True)

            # ---- v2t: mask-folded indicator matmul over t; rhs is [ES|E] ----
            ps_v = psB.tile([8, 2 * BV], F32, tag="v")
            for i in range(NMT):
                for lo, hi in NSL3:
                    nc.tensor.matmul(ps_v[:, lo:hi], ind36m[i],
                                     big[i][:, lo:hi],
                                     start=(i == 0), stop=(i == NMT - 1))

            # ---- vps2 path at [36, x], half-split so DVE/ACT pipeline ----
            fe4 = bigp.tile([8, BV], BF16)
            d4 = smalls.tile([8, B], F32)
            for lo, hi in HALF:
                rdv = smalls.tile([8, 384], F32, name=f"rdv{lo}")
                nc.vector.reciprocal_approx_fast(rdv, ps_v[:8, BV + lo:BV + hi])
                v2t = smalls.tile([8, 384], F32, name=f"v2t{lo}")
                nc.vector.tensor_tensor(v2t, ps_v[:8, lo:hi], rdv, op=MUL)
                nc.scalar.activation(fe4[:, lo:hi], v2t, EXP)
            nc.vector.reduce_sum(d4,
                                 fe4.rearrange("p (g v) -> p g v", v=V),
                                 axis=X)

            # ---- keepalive during the fe4 chain (ps_s slots are dead) ----
            for w in range(2):
                nc.tensor.matmul(ps_s[0][:, 0:512], junk[:, 0:128], junk,
                                 start=True, stop=True)

            # ---- broadcast E4 over t-rows (PE), weight by sp, group-sum ----
            for i in range(NMT):
                ps_w = psA.tile([128, BV], F32, tag="s", name=f"ps_w{i}")
                for lo, hi in WSL:
                    nc.tensor.matmul(ps_w[:, lo:hi],
                                     indW[:, 128 * i:128 * (i + 1)],
                                     fe4[:, lo:hi], start=True, stop=True)
                w4s = sqp.tile([128, BV], BF16, name=f"w4s{i}")
                hun = smalls.tile([128, B], F32, name=f"hun{i}")
                for lo, hi in HALF:
                    nc.vector.tensor_tensor(w4s[:, lo:hi], ps_w[:, lo:hi],
                                            sp[i][:, lo:hi], op=MUL)
                nc.vector.reduce_sum(hun,
                                     w4s.rearrange("p (g v) -> p g v", v=V),
                                     axis=X)
                nc.vector.tensor_tensor(rhs_f[i][:, :B], rhs_f[i][:, B:],
                                        hun, op=MUL)

            ps_o = psB.tile([8, 128], F32, tag="j")
            for i in range(NMT):
                nc.tensor.matmul(ps_o, ind36[:, 8 * i:8 * (i + 1)], rhs_f[i],
                                 start=(i == 0), stop=(i == NMT - 1))
            d4t = smalls.tile([8, B], F32)
            nc.vector.tensor_scalar_mul(d4t, d4, TAU)
            dd = smalls.tile([8, B], F32)
            nc.vector.tensor_tensor(dd, ps_o[:8, B:], d4t, op=MUL)
            rdd = smalls.tile([8, B], F32)
            nc.vector.reciprocal_approx_fast(rdd, dd)
            outw = smalls.tile([8, B], F32)
            nc.vector.tensor_tensor(outw, ps_o[:8, :B], rdd, op=MUL)
            nc.sync.dma_start(out=out_d[:, :], in_=outw[:, :])

    nc.compile()
    return nc


_NC_CACHE = None


def _get_program():
    global _NC_CACHE
    if _NC_CACHE is None:
        _NC_CACHE = _build_program()
    return _NC_CACHE


def _make_in_maps(text_feat, video_feat, text_mask):
    # vT packed k-major: vT_b[p, k*BV + c] = video[(b,v)=c, d=128k+p]
    vflat = video_feat.reshape(BV, D).astype(ml_dtypes.bfloat16)
    vT_b = np.ascontiguousarray(
        vflat.T.reshape(NKT, 128, BV).transpose(1, 0, 2).reshape(128, NKT * BV))
    # ind36 slice i: column 4i + p//T is the block indicator; rows are
    # compact (4 per M-tile, 8 total) so every psum row is live.
    ind36 = np.zeros((128, NMT * 8), np.float32)
    for i in range(NMT):
        for p in range(128):
            ind36[p, 8 * i + 4 * i + p // T] = 1.0
    cpack = np.ones((128, NMT * 8 + 1), ml_dtypes.bfloat16)
    cpack[:, :NMT * 8] = ind36.astype(ml_dtypes.bfloat16)
    # indW slice i: [8, 128] with indW[r, p] = (r == 4i + p//T)
    indW = np.zeros((8, NMT * 128), ml_dtypes.bfloat16)
    for i in range(NMT):
        for p in range(128):
            indW[4 * i + p // T, 128 * i + p] = 1.0
    in_maps = []
    for c in range(NCORES):
        tsl = text_feat[c * AL:(c + 1) * AL].reshape(AT, D) \
            .astype(ml_dtypes.bfloat16)
        tT_b = np.ascontiguousarray(
            tsl.T.reshape(NKT, 128, AT).transpose(1, 0, 2)
            .reshape(128, NKT * AT))
        mask2 = np.ascontiguousarray(
            text_mask[c * AL:(c + 1) * AL].reshape(NMT, 128).T
            .astype(np.float32))
        in_maps.append({
            "tT": tT_b,
            "vT": vT_b,
            "mask": mask2,
            "cpack": cpack,
            "indW": indW,
        })
    return in_maps


def kernel(text_feat, video_feat, text_mask, _trace=False):
    text_feat = np.asarray(text_feat, dtype=np.float32)
    video_feat = np.asarray(video_feat, dtype=np.float32)
    text_mask = np.asarray(text_mask)
    nc = _get_program()
    in_maps = _make_in_maps(text_feat, video_feat, text_mask)
    res = run_bass_kernel_spmd(nc, in_maps, core_ids=list(range(NCORES)),
                               trace=_trace)
    out = np.concatenate([res.results[c]["out"] for c in range(NCORES)], axis=0)
    if _trace:
        kernel.last_exec_time_ns = res.exec_time_ns
        kernel.last_results = res
    return out


# revision 22
# speedup vs baseline: 1.0217x; 1.0217x over previous
"""Trainium2 Bass kernel for the DCM sparse-attention problem (v4, bf16).

Same math restructure as the baseline (S-matrix collapse: every softmax
aggregation is a weighted reduction of S[(a,t),(b,v)] = <t,v>/|t||v|),
plus:

- All matmuls in bf16 (1 cyc/row; DMA bytes halved vs f32).
- Video-norm fold AFTER the S matmul (S matmuls start as soon as tiles
  land); tau and r_t fold into that same scalar_tensor_tensor pass, the
  text mask becomes the E-exponential's per-partition scale, and the
  final division absorbs the leftover tau.
- rsqrt = exp(-0.5*ln(x)); Ln and Exp are steered into the one PWP
  table set that holds both, so there is a single hoisted table load
  and no mid-pipeline table switch.
- Warm-up/keepalive junk matmuls hold the PE HAM clock gate open across
  the DMA window and the elementwise mid-section.
- The mid-section is issued in (M-tile, column-half) streams so DVE and
  ACT pipeline instead of serializing on full-width tensors.
- GpSimd does only the mask/const DMAs (SWDGE) and the one rv
  partition-broadcast; its tensor ops are too slow (drains).

Each of the 8 cores handles 8 of the 64 text rows (A-sharded, video
replicated).
"""

import sys

sys.path.insert(0, "/opt/trn_rl_repo")

import ml_dtypes
import numpy as np

import concourse.bass as bass
import concourse.bacc as bacc
import concourse.hw_specs as hw_specs
import concourse.tile as tile
from concourse import mybir
from concourse.bass_utils import run_bass_kernel_spmd

TAU = 100.0
A, T, B, V, D = 64, 32, 64, 12, 512
NCORES = 8
AL = A // NCORES          # a's per core = 8
AT = AL * T               # (a,t) rows per core = 256
BV = B * V                # (b,v) cols = 768
NMT = AT // 128           # M-tiles over (a,t) = 2
NKT = D // 128            # K-tiles over d = 4
APB = 128 // T            # a's per M-tile = 4
F32 = mybir.dt.float32
BF16 = mybir.dt.bfloat16
EXP = mybir.ActivationFunctionType.Exp
LN = mybir.ActivationFunctionType.Ln
MUL = mybir.AluOpType.mult
X = mybir.AxisListType.X
NSL = [(0, 512), (512, 768)]                   # bank-aligned slices of 768
NSL3 = [(0, 512), (512, 1024), (1024, 1536)]   # ... of 1536
HALF = [(0, 384), (384, 768)]                  # group-aligned halves
WSL = [(0, 384), (384, 512), (512, 768)]       # bank-safe W4 chunks
NWARM = 5

_JOINT = "natural_log_exp_and_others"
_orig_gat = hw_specs.get_activation_tables


def _gat(arch):
    """Steer Ln and Exp to the one table set containing both, so the
    activation-table pass emits a single load instead of three.  Set ids
    are positional, so entries are filtered in place, never reordered."""
    tables = _orig_gat(arch)
    if _JOINT in tables:
        for name, funcs in tables.items():
            if name != _JOINT:
                funcs.discard(LN)
                funcs.discard(EXP)
    return tables


bacc.get_activation_tables = _gat


def _build_program():
    nc = bacc.Bacc("TRN2", target_bir_lowering=False)

    tT_d = nc.declare_dram_parameter("tT", [128, NKT * AT], BF16, isOutput=False)
    vT_d = nc.declare_dram_parameter("vT", [128, NKT * BV], BF16, isOutput=False)
    mask_d = nc.declare_dram_parameter("mask", [128, NMT], F32, isOutput=False)
    cpack_d = nc.declare_dram_parameter("cpack", [128, NMT * 8 + 1], BF16,
                                        isOutput=False)
    indW_d = nc.declare_dram_parameter("indW", [8, NMT * 128], BF16,
                                       isOutput=False)
    out_d = nc.declare_dram_parameter("out", [AL, B], F32, isOutput=True)

    with tile.TileContext(nc) as tc:
        with (
            tc.tile_pool(name="consts", bufs=1) as consts,
            tc.tile_pool(name="inputs", bufs=1) as inputs,
            tc.tile_pool(name="sq", bufs=1) as sqp,
            tc.tile_pool(name="big", bufs=1) as bigp,
            tc.tile_pool(name="smalls", bufs=1) as smalls,
            tc.tile_pool(name="psA", bufs=2, space="PSUM") as psA,
            tc.tile_pool(name="psB", bufs=1, space="PSUM") as psB,
        ):
            # ---- PE warm-up: junk matmuls feed the HAM activity monitor
            # during the DMA window so real matmuls run at released clock ----
            junk = consts.tile([128, 512], BF16)
            nc.vector.memset(junk, 1.0)
            ps_warm = psA.tile([128, 512], F32, tag="s")
            for w in range(NWARM):
                nc.tensor.matmul(ps_warm, junk[:, 0:128], junk,
                                 start=True, stop=True)

            # ---- input DMAs: vT per-k on sync, tT halves on scalar,
            # consts/mask on the idle gpsimd SWDGE queue ----
            vT = inputs.tile([128, NKT * BV], BF16)
            tT = inputs.tile([128, NKT * AT], BF16)
            for k in range(2):
                nc.sync.dma_start(out=vT[:, k * BV:(k + 1) * BV],
                                  in_=vT_d[:, k * BV:(k + 1) * BV])
            nc.scalar.dma_start(out=tT[:, :2 * AT], in_=tT_d[:, :2 * AT])
            nc.scalar.dma_start(out=tT[:, 2 * AT:], in_=tT_d[:, 2 * AT:])
            nc.scalar.dma_start(out=vT[:, 3 * BV:], in_=vT_d[:, 3 * BV:])
            maskt = consts.tile([128, NMT], F32)
            nc.gpsimd.dma_start(out=maskt, in_=mask_d[:, :])
            nc.gpsimd.dma_start(out=vT[:, 2 * BV:3 * BV],
                                in_=vT_d[:, 2 * BV:3 * BV])
            cpack = consts.tile([128, NMT * 8 + 1], BF16)
            nc.gpsimd.dma_start(out=cpack, in_=cpack_d[:, :])
            indW = consts.tile([8, NMT * 128], BF16)
            nc.gpsimd.dma_start(out=indW, in_=indW_d[:, :])
            ind36 = cpack[:, :NMT * 8]
            onesc = cpack[:, NMT * 8:]

            # ---- per-k: square (DVE), norm ones-matmuls, then S matmuls;
            # each k-tile is consumed as its DMA lands ----
            sqv = sqp.tile([128, NKT * BV], BF16)
            sqt = sqp.tile([128, NKT * AT], BF16)
            ps_nv = psB.tile([1, BV], F32, tag="v")
            ps_nt = psB.tile([1, AT], F32, tag="j")
            ps_s = [psA.tile([128, BV], F32, tag="s", name=f"ps_s{i}")
                    for i in range(NMT)]
            for k in range(NKT):
                nc.vector.tensor_tensor(sqv[:, k * BV:(k + 1) * BV],
                                        vT[:, k * BV:(k + 1) * BV],
                                        vT[:, k * BV:(k + 1) * BV], op=MUL)
                for lo, hi in NSL:
                    nc.tensor.matmul(ps_nv[:, lo:hi], onesc,
                                     sqv[:, k * BV + lo:k * BV + hi],
                                     start=(k == 0), stop=(k == NKT - 1))
                if k % 2 == 0:
                    nc.vector.tensor_tensor(
                        sqt[:, k * AT:(k + 2) * AT],
                        tT[:, k * AT:(k + 2) * AT],
                        tT[:, k * AT:(k + 2) * AT], op=MUL)
                nc.tensor.matmul(ps_nt, onesc,
                                 sqt[:, k * AT:(k + 1) * AT],
                                 start=(k == 0), stop=(k == NKT - 1))
                for i in range(NMT):
                    for lo, hi in NSL:
                        nc.tensor.matmul(
                            ps_s[i][:, lo:hi],
                            tT[:, k * AT + 128 * i:k * AT + 128 * (i + 1)],
                            vT[:, k * BV + lo:k * BV + hi],
                            start=(k == 0), stop=(k == NKT - 1))

            # ---- norms: rsqrt = exp(-0.5*ln(ss)); v-row first so the
            # broadcast starts as early as possible ----
            lss = smalls.tile([1, BV + AT], F32)
            rr = smalls.tile([1, BV + AT], F32)
            rv_bc = bigp.tile([128, BV], F32)
            for lo, hi in HALF:
                nc.scalar.activation(lss[:, lo:hi], ps_nv[:, lo:hi], LN)
                nc.scalar.activation(rr[:, lo:hi], lss[:, lo:hi], EXP,
                                     scale=-0.5)
                nc.gpsimd.partition_broadcast(rv_bc[:, lo:hi], rr[:, lo:hi],
                                              channels=128)
            nc.scalar.activation(lss[:, BV:], ps_nt, LN)
            nc.scalar.activation(rr[:, BV:], lss[:, BV:], EXP, scale=-0.5)
            ident = consts.tile([1, 1], F32)
            nc.vector.memset(ident, 1.0)
            tau_rt = [smalls.tile([128, 1], F32, name=f"tau_rt{i}")
                      for i in range(NMT)]
            ind36m = [smalls.tile([128, 8], BF16, name=f"ind36m{i}")
                      for i in range(NMT)]
            for i in range(NMT):
                ps_tr = psB.tile([128, 1], F32, tag="j", name=f"ps_tr{i}")
                nc.tensor.transpose(ps_tr,
                                    rr[:, BV + 128 * i:BV + 128 * (i + 1)],
                                    ident)
                nc.vector.tensor_scalar_mul(tau_rt[i], ps_tr, TAU)
                nc.vector.tensor_scalar_mul(ind36m[i],
                                            ind36[:, 8 * i:8 * (i + 1)],
                                            maskt[:, i:i + 1])

            # ---- mid section in (i, half) streams: sp = tau*r_t*rv*S from
            # PSUM, E = exp(mask*sp), ES = sp*E, then grouped reduces ----
            sp = [bigp.tile([128, BV], BF16, name=f"sp{i}") for i in range(NMT)]
            big = [bigp.tile([128, 2 * BV], BF16, name=f"big{i}")
                   for i in range(NMT)]
            rhs_f = [smalls.tile([128, 128], BF16, name=f"rhs_f{i}")
                     for i in range(NMT)]
            red = [smalls.tile([128, 128], F32, name=f"red{i}")
                   for i in range(NMT)]
            for i in range(NMT):
                for lo, hi in HALF:
                    nc.vector.scalar_tensor_tensor(
                        sp[i][:, lo:hi], ps_s[i][:, lo:hi], tau_rt[i],
                        rv_bc[:, lo:hi], op0=MUL, op1=MUL)
                    nc.scalar.activation(big[i][:, BV + lo:BV + hi],
                                         sp[i][:, lo:hi], EXP,
                                         scale=maskt[:, i:i + 1])
            for i in range(NMT):
                nc.vector.reduce_sum(red[i][:, B:],
                                     big[i][:, BV:].rearrange(
                                         "p (g v) -> p g v", v=V), axis=X)
                for lo, hi in HALF:
                    nc.vector.tensor_tensor(big[i][:, lo:hi], sp[i][:, lo:hi],
                                            big[i][:, BV + lo:BV + hi], op=MUL)
                nc.vector.reduce_sum(red[i][:, :B],
                                     big[i][:, :BV].rearrange(
                                         "p (g v) -> p g v", v=V), axis=X)
            for i in range(NMT):
                rdn = smalls.tile([128, B], F32, name=f"rdn{i}")
                nc.vector.reciprocal_approx_fast(rdn, red[i][:, B:])
                t2v = smalls.tile([128, B], F32, name=f"t2v{i}")
                nc.vector.tensor_tensor(t2v, red[i][:, :B], rdn, op=MUL)
                nc.scalar.activation(rhs_f[i][:, B:], t2v, EXP)

            # ---- PE keepalive while DVE/ACT chew the mid-section ----
            for w in range(3):
                nc.tensor.matmul(ps_nv[:, 0:512], onesc, junk,
                                 start=True, stop=True)

            # ---- v2t: mask-folded indicator matmul over t; rhs is [ES|E] ----
            ps_v = psB.tile([8, 2 * BV], F32, tag="v")
            for i in range(NMT):
                for lo, hi in NSL3:
                    nc.tensor.matmul(ps_v[:, lo:hi], ind36m[i],
                                     big[i][:, lo:hi],
                                     start=(i == 0), stop=(i == NMT - 1))

            # ---- vps2 path at [36, x], half-split so DVE/ACT pipeline ----
            fe4 = bigp.tile([8, BV], BF16)
            d4 = smalls.tile([8, B], F32)
            for lo, hi in HALF:
                rdv = smalls.tile([8, 384], F32, name=f"rdv{lo}")
                nc.vector.reciprocal_approx_fast(rdv, ps_v[:8, BV + lo:BV + hi])
                v2t = smalls.tile([8, 384], F32, name=f"v2t{lo}")
                nc.vector.tensor_tensor(v2t, ps_v[:8, lo:hi], rdv, op=MUL)
                nc.scalar.activation(fe4[:, lo:hi], v2t, EXP)
            nc.vector.reduce_sum(d4,
                                 fe4.rearrange("p (g v) -> p g v", v=V),
                                 axis=X)

            # ---- keepalive during the fe4 chain (ps_s slots are dead) ----
            for w in range(2):
                nc.tensor.matmul(ps_s[0][:, 0:512], junk[:, 0:128], junk,
                                 start=True, stop=True)

            # ---- broadcast E4 over t-rows (PE), weight by sp, group-sum ----
            for i in range(NMT):
                ps_w = psA.tile([128, BV], F32, tag="s", name=f"ps_w{i}")
                for lo, hi in WSL:
                    nc.tensor.matmul(ps_w[:, lo:hi],
                                     indW[:, 128 * i:128 * (i + 1)],
                                     fe4[:, lo:hi], start=True, stop=True)
                w4s = sqp.tile([128, BV], BF16, name=f"w4s{i}")
                hun = smalls.tile([128, B], F32, name=f"hun{i}")
                for lo, hi in HALF:
                    nc.vector.tensor_tensor(w4s[:, lo:hi], ps_w[:, lo:hi],
                                            sp[i][:, lo:hi], op=MUL)
                nc.vector.reduce_sum(hun,
                                     w4s.rearrange("p (g v) -> p g v", v=V),
                                     axis=X)
                nc.vector.tensor_tensor(rhs_f[i][:, :B], rhs_f[i][:, B:],
                                        hun, op=MUL)

            ps_o = psB.tile([8, 128], F32, tag="j")
            for i in range(NMT):
                nc.tensor.matmul(ps_o, ind36[:, 8 * i:8 * (i + 1)], rhs_f[i],
                                 start=(i == 0), stop=(i == NMT - 1))
            d4t = smalls.tile([8, B], F32)
            nc.vector.tensor_scalar_mul(d4t, d4, TAU)
            dd = smalls.tile([8, B], F32)
            nc.vector.tensor_tensor(dd, ps_o[:8, B:], d4t, op=MUL)
            rdd = smalls.tile([8, B], F32)
            nc.vector.reciprocal_approx_fast(rdd, dd)
            outw = smalls.tile([8, B], F32)
            nc.vector.tensor_tensor(outw, ps_o[:8, :B], rdd, op=MUL)
            nc.sync.dma_start(out=out_d[:, :], in_=outw[:, :])

    nc.compile()
    return nc


_NC_CACHE = None


def _get_program():
    global _NC_CACHE
    if _NC_CACHE is None:
        _NC_CACHE = _build_program()
    return _NC_CACHE


def _make_in_maps(text_feat, video_feat, text_mask):
    # vT packed k-major: vT_b[p, k*BV + c] = video[(b,v)=c, d=128k+p]
    vflat = video_feat.reshape(BV, D).astype(ml_dtypes.bfloat16)
    vT_b = np.ascontiguousarray(
        vflat.T.reshape(NKT, 128, BV).transpose(1, 0, 2).reshape(128, NKT * BV))
    # ind36 slice i: column 4i + p//T is the block indicator; rows are
    # compact (4 per M-tile, 8 total) so every psum row is live.
    ind36 = np.zeros((128, NMT * 8), np.float32)
    for i in range(NMT):
        for p in range(128):
            ind36[p, 8 * i + 4 * i + p // T] = 1.0
    cpack = np.ones((128, NMT * 8 + 1), ml_dtypes.bfloat16)
    cpack[:, :NMT * 8] = ind36.astype(ml_dtypes.bfloat16)
    # indW slice i: [8, 128] with indW[r, p] = (r == 4i + p//T)
    indW = np.zeros((8, NMT * 128), ml_dtypes.bfloat16)
    for i in range(NMT):
        for p in range(128):
            indW[4 * i + p // T, 128 * i + p] = 1.0
    in_maps = []
    for c in range(NCORES):
        tsl = text_feat[c * AL:(c + 1) * AL].reshape(AT, D) \
            .astype(ml_dtypes.bfloat16)
        tT_b = np.ascontiguousarray(
            tsl.T.reshape(NKT, 128, AT).transpose(1, 0, 2)
            .reshape(128, NKT * AT))
        mask2 = np.ascontiguousarray(
            text_mask[c * AL:(c + 1) * AL].reshape(NMT, 128).T
            .astype(np.float32))
        in_maps.append({
            "tT": tT_b,
            "vT": vT_b,
            "mask": mask2,
            "cpack": cpack,
            "indW": indW,
        })
    return in_maps


def kernel(text_feat, video_feat, text_mask, _trace=False):
    text_feat = np.asarray(text_feat, dtype=np.float32)
    video_feat = np.asarray(video_feat, dtype=np.float32)
    text_mask = np.asarray(text_mask)
    nc = _get_program()
    in_maps = _make_in_maps(text_feat, video_feat, text_mask)
    res = run_bass_kernel_spmd(nc, in_maps, core_ids=list(range(NCORES)),
                               trace=_trace)
    out = np.concatenate([res.results[c]["out"] for c in range(NCORES)], axis=0)
    if _trace:
        kernel.last_exec_time_ns = res.exec_time_ns
        kernel.last_results = res
    return out


# revision 23
# speedup vs baseline: 1.0276x; 1.0057x over previous
"""Trainium2 Bass kernel for the DCM sparse-attention problem (v4, bf16).

Same math restructure as the baseline (S-matrix collapse: every softmax
aggregation is a weighted reduction of S[(a,t),(b,v)] = <t,v>/|t||v|),
plus:

- All matmuls in bf16 (1 cyc/row; DMA bytes halved vs f32).
- Video-norm fold AFTER the S matmul (S matmuls start as soon as tiles
  land); tau and r_t fold into that same scalar_tensor_tensor pass, the
  text mask becomes the E-exponential's per-partition scale, and the
  final division absorbs the leftover tau.
- rsqrt = exp(-0.5*ln(x)); Ln and Exp are steered into the one PWP
  table set that holds both, so there is a single hoisted table load
  and no mid-pipeline table switch.
- Warm-up/keepalive junk matmuls hold the PE HAM clock gate open across
  the DMA window and the elementwise mid-section.
- The mid-section is issued in (M-tile, column-half) streams so DVE and
  ACT pipeline instead of serializing on full-width tensors.
- GpSimd does only the mask/const DMAs (SWDGE) and the one rv
  partition-broadcast; its tensor ops are too slow (drains).

Each of the 8 cores handles 8 of the 64 text rows (A-sharded, video
replicated).
"""

import sys

sys.path.insert(0, "/opt/trn_rl_repo")

import ml_dtypes
import numpy as np

import concourse.bass as bass
import concourse.bacc as bacc
import concourse.hw_specs as hw_specs
import concourse.tile as tile
from concourse import mybir
from concourse.bass_utils import run_bass_kernel_spmd

TAU = 100.0
A, T, B, V, D = 64, 32, 64, 12, 512
NCORES = 8
AL = A // NCORES          # a's per core = 8
AT = AL * T               # (a,t) rows per core = 256
BV = B * V                # (b,v) cols = 768
NMT = AT // 128           # M-tiles over (a,t) = 2
NKT = D // 128            # K-tiles over d = 4
APB = 128 // T            # a's per M-tile = 4
F32 = mybir.dt.float32
BF16 = mybir.dt.bfloat16
EXP = mybir.ActivationFunctionType.Exp
LN = mybir.ActivationFunctionType.Ln
MUL = mybir.AluOpType.mult
X = mybir.AxisListType.X
NSL = [(0, 512), (512, 768)]                   # bank-aligned slices of 768
NSL3 = [(0, 512), (512, 1024), (1024, 1536)]   # ... of 1536
HALF = [(0, 384), (384, 768)]                  # group-aligned halves
WSL = [(0, 384), (384, 512), (512, 768)]       # bank-safe W4 chunks
NWARM = 5

_JOINT = "natural_log_exp_and_others"
_orig_gat = hw_specs.get_activation_tables


def _gat(arch):
    """Steer Ln and Exp to the one table set containing both, so the
    activation-table pass emits a single load instead of three.  Set ids
    are positional, so entries are filtered in place, never reordered."""
    tables = _orig_gat(arch)
    if _JOINT in tables:
        for name, funcs in tables.items():
            if name != _JOINT:
                funcs.discard(LN)
                funcs.discard(EXP)
    return tables


bacc.get_activation_tables = _gat


def _build_program():
    nc = bacc.Bacc("TRN2", target_bir_lowering=False)

    tT_d = nc.declare_dram_parameter("tT", [128, NKT * AT], BF16, isOutput=False)
    vT_d = nc.declare_dram_parameter("vT", [128, NKT * BV], BF16, isOutput=False)
    mask_d = nc.declare_dram_parameter("mask", [128, NMT], F32, isOutput=False)
    cpack_d = nc.declare_dram_parameter("cpack", [128, NMT * 8 + 1], BF16,
                                        isOutput=False)
    indW_d = nc.declare_dram_parameter("indW", [8, NMT * 128], BF16,
                                       isOutput=False)
    out_d = nc.declare_dram_parameter("out", [AL, B], F32, isOutput=True)

    with tile.TileContext(nc) as tc:
        with (
            tc.tile_pool(name="consts", bufs=1) as consts,
            tc.tile_pool(name="inputs", bufs=1) as inputs,
            tc.tile_pool(name="sq", bufs=1) as sqp,
            tc.tile_pool(name="big", bufs=1) as bigp,
            tc.tile_pool(name="smalls", bufs=1) as smalls,
            tc.tile_pool(name="psA", bufs=2, space="PSUM") as psA,
            tc.tile_pool(name="psB", bufs=1, space="PSUM") as psB,
        ):
            # ---- PE warm-up: junk matmuls feed the HAM activity monitor
            # during the DMA window so real matmuls run at released clock ----
            junk = consts.tile([128, 512], BF16)
            nc.vector.memset(junk, 1.0)
            ps_warm = psA.tile([128, 512], F32, tag="s")
            for w in range(NWARM):
                nc.tensor.matmul(ps_warm, junk[:, 0:128], junk,
                                 start=True, stop=True)

            # ---- input DMAs: vT per-k on sync, tT halves on scalar,
            # consts/mask on the idle gpsimd SWDGE queue ----
            vT = inputs.tile([128, NKT * BV], BF16)
            tT = inputs.tile([128, NKT * AT], BF16)
            for k in range(2):
                nc.sync.dma_start(out=vT[:, k * BV:(k + 1) * BV],
                                  in_=vT_d[:, k * BV:(k + 1) * BV])
            nc.scalar.dma_start(out=tT[:, :2 * AT], in_=tT_d[:, :2 * AT])
            nc.scalar.dma_start(out=tT[:, 2 * AT:], in_=tT_d[:, 2 * AT:])
            nc.scalar.dma_start(out=vT[:, 3 * BV:], in_=vT_d[:, 3 * BV:])
            maskt = consts.tile([128, NMT], F32)
            nc.gpsimd.dma_start(out=maskt, in_=mask_d[:, :])
            nc.gpsimd.dma_start(out=vT[:, 2 * BV:3 * BV],
                                in_=vT_d[:, 2 * BV:3 * BV])
            cpack = consts.tile([128, NMT * 8 + 1], BF16)
            nc.gpsimd.dma_start(out=cpack, in_=cpack_d[:, :])
            indW = consts.tile([8, NMT * 128], BF16)
            nc.gpsimd.dma_start(out=indW, in_=indW_d[:, :])
            ind36 = cpack[:, :NMT * 8]
            onesc = cpack[:, NMT * 8:]

            # ---- per-k: square (DVE), norm ones-matmuls, then S matmuls;
            # each k-tile is consumed as its DMA lands ----
            sqv = sqp.tile([128, NKT * BV], BF16)
            sqt = sqp.tile([128, NKT * AT], BF16)
            ps_nv = psB.tile([1, BV], F32, tag="v")
            ps_nt = psB.tile([1, AT], F32, tag="j")
            ps_s = [psA.tile([128, BV], F32, tag="s", name=f"ps_s{i}")
                    for i in range(NMT)]
            def _norm_mms(k):
                for lo, hi in NSL:
                    nc.tensor.matmul(ps_nv[:, lo:hi], onesc,
                                     sqv[:, k * BV + lo:k * BV + hi],
                                     start=(k == 0), stop=(k == NKT - 1))
                nc.tensor.matmul(ps_nt, onesc,
                                 sqt[:, k * AT:(k + 1) * AT],
                                 start=(k == 0), stop=(k == NKT - 1))

            def _s_mms(k):
                for i in range(NMT):
                    for lo, hi in NSL:
                        nc.tensor.matmul(
                            ps_s[i][:, lo:hi],
                            tT[:, k * AT + 128 * i:k * AT + 128 * (i + 1)],
                            vT[:, k * BV + lo:k * BV + hi],
                            start=(k == 0), stop=(k == NKT - 1))

            for k in range(NKT):
                nc.vector.tensor_tensor(sqv[:, k * BV:(k + 1) * BV],
                                        vT[:, k * BV:(k + 1) * BV],
                                        vT[:, k * BV:(k + 1) * BV], op=MUL)
                if k % 2 == 0:
                    nc.vector.tensor_tensor(
                        sqt[:, k * AT:(k + 2) * AT],
                        tT[:, k * AT:(k + 2) * AT],
                        tT[:, k * AT:(k + 2) * AT], op=MUL)
                if k < 2:
                    _norm_mms(k)
                    _s_mms(k)
            # norm k2/k3 hoisted ahead of S-k2/k3: their sqv inputs land
            # ~2us before the PE reaches this point, so the rv chain
            # (ln/exp/bcast) starts as early as the data allows
            _norm_mms(2)
            _norm_mms(3)
            _s_mms(2)
            _s_mms(3)

            # ---- norms: rsqrt = exp(-0.5*ln(ss)); v-row first so the
            # broadcast starts as early as possible ----
            lss = smalls.tile([1, BV + AT], F32)
            rr = smalls.tile([1, BV + AT], F32)
            rv_bc = bigp.tile([128, BV], F32)
            for lo, hi in HALF:
                nc.scalar.activation(lss[:, lo:hi], ps_nv[:, lo:hi], LN)
                nc.scalar.activation(rr[:, lo:hi], lss[:, lo:hi], EXP,
                                     scale=-0.5)
                nc.gpsimd.partition_broadcast(rv_bc[:, lo:hi], rr[:, lo:hi],
                                              channels=128)
            nc.scalar.activation(lss[:, BV:], ps_nt, LN)
            nc.scalar.activation(rr[:, BV:], lss[:, BV:], EXP, scale=-0.5)
            ident = consts.tile([1, 1], F32)
            nc.vector.memset(ident, 1.0)
            tau_rt = [smalls.tile([128, 1], F32, name=f"tau_rt{i}")
                      for i in range(NMT)]
            ind36m = [smalls.tile([128, 8], BF16, name=f"ind36m{i}")
                      for i in range(NMT)]
            for i in range(NMT):
                ps_tr = psB.tile([128, 1], F32, tag="j", name=f"ps_tr{i}")
                nc.tensor.transpose(ps_tr,
                                    rr[:, BV + 128 * i:BV + 128 * (i + 1)],
                                    ident)
                nc.vector.tensor_scalar_mul(tau_rt[i], ps_tr, TAU)
                nc.vector.tensor_scalar_mul(ind36m[i],
                                            ind36[:, 8 * i:8 * (i + 1)],
                                            maskt[:, i:i + 1])

            # ---- mid section in (i, half) streams: sp = tau*r_t*rv*S from
            # PSUM, E = exp(mask*sp), ES = sp*E, then grouped reduces ----
            sp = [bigp.tile([128, BV], BF16, name=f"sp{i}") for i in range(NMT)]
            big = [bigp.tile([128, 2 * BV], BF16, name=f"big{i}")
                   for i in range(NMT)]
            rhs_f = [smalls.tile([128, 128], BF16, name=f"rhs_f{i}")
                     for i in range(NMT)]
            red = [smalls.tile([128, 128], F32, name=f"red{i}")
                   for i in range(NMT)]
            for i in range(NMT):
                for lo, hi in HALF:
                    nc.vector.scalar_tensor_tensor(
                        sp[i][:, lo:hi], ps_s[i][:, lo:hi], tau_rt[i],
                        rv_bc[:, lo:hi], op0=MUL, op1=MUL)
                    nc.scalar.activation(big[i][:, BV + lo:BV + hi],
                                         sp[i][:, lo:hi], EXP,
                                         scale=maskt[:, i:i + 1])
            for i in range(NMT):
                nc.vector.reduce_sum(red[i][:, B:],
                                     big[i][:, BV:].rearrange(
                                         "p (g v) -> p g v", v=V), axis=X)
                for lo, hi in HALF:
                    nc.vector.tensor_tensor(big[i][:, lo:hi], sp[i][:, lo:hi],
                                            big[i][:, BV + lo:BV + hi], op=MUL)
                nc.vector.reduce_sum(red[i][:, :B],
                                     big[i][:, :BV].rearrange(
                                         "p (g v) -> p g v", v=V), axis=X)
            for i in range(NMT):
                rdn = smalls.tile([128, B], F32, name=f"rdn{i}")
                nc.vector.reciprocal_approx_fast(rdn, red[i][:, B:])
                t2v = smalls.tile([128, B], F32, name=f"t2v{i}")
                nc.vector.tensor_tensor(t2v, red[i][:, :B], rdn, op=MUL)
                nc.scalar.activation(rhs_f[i][:, B:], t2v, EXP)

            # ---- PE keepalive while DVE/ACT chew the mid-section ----
            for w in range(3):
                nc.tensor.matmul(ps_nv[:, 0:512], onesc, junk,
                                 start=True, stop=True)

            # ---- v2t: mask-folded indicator matmul over t; rhs is [ES|E] ----
            ps_v = psB.tile([8, 2 * BV], F32, tag="v")
            for i in range(NMT):
                order = NSL3 if i == 0 else [NSL3[1], NSL3[2], NSL3[0]]
                for lo, hi in order:
                    nc.tensor.matmul(ps_v[:, lo:hi], ind36m[i],
                                     big[i][:, lo:hi],
                                     start=(i == 0), stop=(i == NMT - 1))

            # ---- vps2 path at [36, x], half-split so DVE/ACT pipeline ----
            fe4 = bigp.tile([8, BV], BF16)
            d4 = smalls.tile([8, B], F32)
            for lo, hi in HALF:
                rdv = smalls.tile([8, 384], F32, name=f"rdv{lo}")
                nc.vector.reciprocal_approx_fast(rdv, ps_v[:8, BV + lo:BV + hi])
                v2t = smalls.tile([8, 384], F32, name=f"v2t{lo}")
                nc.vector.tensor_tensor(v2t, ps_v[:8, lo:hi], rdv, op=MUL)
                nc.scalar.activation(fe4[:, lo:hi], v2t, EXP)
            nc.vector.reduce_sum(d4,
                                 fe4.rearrange("p (g v) -> p g v", v=V),
                                 axis=X)

            # ---- keepalive during the fe4 chain (ps_s slots are dead) ----
            for w in range(2):
                nc.tensor.matmul(ps_s[0][:, 0:512], junk[:, 0:128], junk,
                                 start=True, stop=True)

            # ---- broadcast E4 over t-rows (PE), weight by sp, group-sum ----
            for i in range(NMT):
                ps_w = psA.tile([128, BV], F32, tag="s", name=f"ps_w{i}")
                for lo, hi in WSL:
                    nc.tensor.matmul(ps_w[:, lo:hi],
                                     indW[:, 128 * i:128 * (i + 1)],
                                     fe4[:, lo:hi], start=True, stop=True)
                w4s = sqp.tile([128, BV], BF16, name=f"w4s{i}")
                hun = smalls.tile([128, B], F32, name=f"hun{i}")
                for lo, hi in HALF:
                    nc.vector.tensor_tensor(w4s[:, lo:hi], ps_w[:, lo:hi],
                                            sp[i][:, lo:hi], op=MUL)
                nc.vector.reduce_sum(hun,
                                     w4s.rearrange("p (g v) -> p g v", v=V),
                                     axis=X)
                nc.vector.tensor_tensor(rhs_f[i][:, :B], rhs_f[i][:, B:],
                                        hun, op=MUL)

            ps_o = psB.tile([8, 128], F32, tag="j")
            for i in range(NMT):
                nc.tensor.matmul(ps_o, ind36[:, 8 * i:8 * (i + 1)], rhs_f[i],
                                 start=(i == 0), stop=(i == NMT - 1))
            d4t = smalls.tile([8, B], F32)
            nc.vector.tensor_scalar_mul(d4t, d4, TAU)
            dd = smalls.tile([8, B], F32)
            nc.vector.tensor_tensor(dd, ps_o[:8, B:], d4t, op=MUL)
            rdd = smalls.tile([8, B], F32)
            nc.vector.reciprocal_approx_fast(rdd, dd)
            outw = smalls.tile([8, B], F32)
            nc.vector.tensor_tensor(outw, ps_o[:8, :B], rdd, op=MUL)
            nc.sync.dma_start(out=out_d[:, :], in_=outw[:, :])

    nc.compile()
    return nc


_NC_CACHE = None


def _get_program():
    global _NC_CACHE
    if _NC_CACHE is None:
        _NC_CACHE = _build_program()
    return _NC_CACHE


def _make_in_maps(text_feat, video_feat, text_mask):
    # vT packed k-major: vT_b[p, k*BV + c] = video[(b,v)=c, d=128k+p]
    vflat = video_feat.reshape(BV, D).astype(ml_dtypes.bfloat16)
    vT_b = np.ascontiguousarray(
        vflat.T.reshape(NKT, 128, BV).transpose(1, 0, 2).reshape(128, NKT * BV))
    # ind36 slice i: column 4i + p//T is the block indicator; rows are
    # compact (4 per M-tile, 8 total) so every psum row is live.
    ind36 = np.zeros((128, NMT * 8), np.float32)
    for i in range(NMT):
        for p in range(128):
            ind36[p, 8 * i + 4 * i + p // T] = 1.0
    cpack = np.ones((128, NMT * 8 + 1), ml_dtypes.bfloat16)
    cpack[:, :NMT * 8] = ind36.astype(ml_dtypes.bfloat16)
    # indW slice i: [8, 128] with indW[r, p] = (r == 4i + p//T)
    indW = np.zeros((8, NMT * 128), ml_dtypes.bfloat16)
    for i in range(NMT):
        for p in range(128):
            indW[4 * i + p // T, 128 * i + p] = 1.0
    in_maps = []
    for c in range(NCORES):
        tsl = text_feat[c * AL:(c + 1) * AL].reshape(AT, D) \
            .astype(ml_dtypes.bfloat16)
        tT_b = np.ascontiguousarray(
            tsl.T.reshape(NKT, 128, AT).transpose(1, 0, 2)
            .reshape(128, NKT * AT))
        mask2 = np.ascontiguousarray(
            text_mask[c * AL:(c + 1) * AL].reshape(NMT, 128).T
            .astype(np.float32))
        in_maps.append({
            "tT": tT_b,
            "vT": vT_b,
            "mask": mask2,
            "cpack": cpack,
            "indW": indW,
        })
    return in_maps


def kernel(text_feat, video_feat, text_mask, _trace=False):
    text_feat = np.asarray(text_feat, dtype=np.float32)
    video_feat = np.asarray(video_feat, dtype=np.float32)
    text_mask = np.asarray(text_mask)
    nc = _get_program()
    in_maps = _make_in_maps(text_feat, video_feat, text_mask)
    res = run_bass_kernel_spmd(nc, in_maps, core_ids=list(range(NCORES)),
                               trace=_trace)
    out = np.concatenate([res.results[c]["out"] for c in range(NCORES)], axis=0)
    if _trace:
        kernel.last_exec_time_ns = res.exec_time_ns
        kernel.last_results = res
    return out
